# revision 5
# baseline (speedup 1.0000x reference)
"""GRU-D forward on 8 Trainium2 NeuronCores (Bass/Tile kernel).

Key algebraic structure exploited:
  - The gates z_t, h~_t depend only on inputs (not on h), so all matmuls are
    parallel over T; only the elementwise blend h = (1-z)h + z*h~ is a
    recurrence, and it maps onto the DVE tensor_tensor_scan instruction
    (state = a*state - c' with a = sigmoid(-pre_z), c' = (a-1)*h~).
  - r_t is computed-but-unused in the reference -> Wr matmul skipped.
  - The xm block of inp = [x_tilde, xm, m] is constant -> folded into biases.

Sharding: data-parallel over batch (64 rows per core); weights replicated.
"""

import os
import sys

import numpy as np

sys.path.insert(0, "/opt/trn_rl_repo")

B, T, D, H = 512, 256, 256, 1024
NC = 8
BL = B // NC  # 64 batch rows per core
SB = 2  # batch elems per sub-batch (x T=256 -> 512 matmul rows)
NSB = BL // SB  # 32 sub-batches
KT = 4  # K tiles of 128 over 2D=512 contraction
HT = 8  # H tiles of 128
P = 128

_nc_cache = None
_last_results = None


def _build_bass():
    global _nc_cache
    if _nc_cache is not None:
        return _nc_cache
    from concourse import bacc, mybir, tile

    nc = bacc.Bacc("TRN2", target_bir_lowering=False, debug=False, num_devices=NC)
    bf16 = mybir.dt.bfloat16
    f32 = mybir.dt.float32
    AF = mybir.ActivationFunctionType
    OP = mybir.AluOpType

    in_d = nc.dram_tensor("inp", [P, NSB, KT, T, SB], bf16, kind="ExternalInput")
    w_d = nc.dram_tensor("w", [P, 2, KT, HT, P], bf16, kind="ExternalInput")
    nbz_d = nc.dram_tensor("nbz", [P, HT], f32, kind="ExternalInput")
    bh_d = nc.dram_tensor("bh", [P, HT], f32, kind="ExternalInput")
    hout_d = nc.dram_tensor("hout", [P, HT, BL], f32, kind="ExternalOutput")

    with tile.TileContext(nc) as tc:
        with (
            tc.tile_pool(name="const", bufs=1) as cpool,
            tc.tile_pool(name="inb", bufs=3) as ipool,
            tc.tile_pool(name="act", bufs=2) as apool,
            tc.tile_pool(name="zps", bufs=2, space="PSUM") as zpool,
            tc.tile_pool(name="hps", bufs=2, space="PSUM") as hpool,
        ):
            w_s = cpool.tile([P, 2, KT, HT, P], bf16)
            nbz_s = cpool.tile([P, HT], f32)
            bh_s = cpool.tile([P, HT], f32)
            hlast = cpool.tile([P, HT, BL], f32)
            nc.sync.dma_start(w_s[:], w_d[:])
            nc.sync.dma_start(nbz_s[:], nbz_d[:])
            nc.sync.dma_start(bh_s[:], bh_d[:])

            for sb in range(NSB):
                in_s = ipool.tile([P, KT, T, SB], bf16, tag="in")
                nc.sync.dma_start(in_s[:], in_d[:, sb])
                a_s = apool.tile([P, HT, T, SB], bf16, tag="a")
                ht_s = apool.tile([P, HT, T, SB], bf16, tag="ht")
                cp_s = apool.tile([P, HT, T, SB], bf16, tag="cp")
                so_s = apool.tile([P, HT, T, SB], f32, tag="so")
                for j in range(HT):
                    ps_z = zpool.tile([P, T, SB], f32, tag="z")
                    ps_h = hpool.tile([P, T, SB], f32, tag="h")
                    for k in range(KT):
                        nc.tensor.matmul(
                            ps_z[:],
                            w_s[:, 0, k, j, :],
                            in_s[:, k],
                            start=(k == 0),
                            stop=(k == KT - 1),
                        )
                    for k in range(KT):
                        nc.tensor.matmul(
                            ps_h[:],
                            w_s[:, 1, k, j, :],
                            in_s[:, k],
                            start=(k == 0),
                            stop=(k == KT - 1),
                        )
                    # a = 1 - z = sigmoid(-(pre_z + bz))
                    nc.scalar.activation(
                        a_s[:, j], ps_z[:], AF.Sigmoid, bias=nbz_s[:, j : j + 1], scale=-1.0
                    )
                    nc.scalar.activation(
                        ht_s[:, j], ps_h[:], AF.Tanh, bias=bh_s[:, j : j + 1], scale=1.0
                    )
                    # c' = (a - 1) * h~   (so that a*h - c' = a*h + (1-a)*h~)
                    nc.vector.scalar_tensor_tensor(
                        cp_s[:, j], a_s[:, j], 1.0, ht_s[:, j], op0=OP.subtract, op1=OP.mult
                    )
                    for b in range(SB):
                        nc.vector.tensor_tensor_scan(
                            so_s[:, j, :, b],
                            a_s[:, j, :, b],
                            cp_s[:, j, :, b],
                            0.0,
                            op0=OP.mult,
                            op1=OP.subtract,
                        )
                    nc.vector.tensor_copy(
                        hlast[:, j, sb * SB : (sb + 1) * SB], so_s[:, j, T - 1, :]
                    )
            nc.sync.dma_start(hout_d[:], hlast[:])
    nc.compile()
    _nc_cache = nc
    return nc


def _prepare_in_maps(X, M, input_means, gamma_x, Wz, bz, Wh, bh):
    import ml_dtypes

    bf16 = ml_dtypes.bfloat16
    X = np.asarray(X, np.float32)
    M = np.asarray(M, np.float32)
    xm = np.asarray(input_means, np.float32)
    gx = np.asarray(gamma_x, np.float32)
    Wz = np.asarray(Wz, np.float32)
    Wh = np.asarray(Wh, np.float32)
    bz = np.asarray(bz, np.float32)
    bhv = np.asarray(bh, np.float32)

    # x_tilde (exact, fp32, handles arbitrary gamma_x / non-binary M)
    g = np.exp(-gx * (1.0 - M))
    x_hat = M * X + (1.0 - M) * xm
    x_tilde = g * x_hat + (1.0 - g) * xm  # [B, T, D]

    # fold the constant xm block into the biases; drop unused Wr entirely
    Wz_eff = np.concatenate([Wz[:, :D], Wz[:, 2 * D :]], axis=1).T  # [2D, H]
    Wh_eff = np.concatenate([Wh[:, :D], Wh[:, 2 * D :]], axis=1).T
    bz_eff = bz + xm @ Wz[:, D : 2 * D].T
    bh_eff = bhv + xm @ Wh[:, D : 2 * D].T

    def wdev(weff):  # [2D, H] -> [P, KT, HT, P]
        return weff.reshape(KT, P, HT, P).transpose(1, 0, 2, 3)

    w_all = np.ascontiguousarray(
        np.stack([wdev(Wz_eff), wdev(Wh_eff)], axis=1)
    ).astype(bf16)  # [P, 2, KT, HT, P]
    nbz_dev = np.ascontiguousarray((-bz_eff).reshape(HT, P).T).astype(np.float32)
    bh_dev = np.ascontiguousarray(bh_eff.reshape(HT, P).T).astype(np.float32)

    in_maps = []
    for c in range(NC):
        xt_c = x_tilde[c * BL : (c + 1) * BL]  # [BL, T, D]
        m_c = M[c * BL : (c + 1) * BL]
        feat = np.concatenate(
            [xt_c.transpose(2, 0, 1), m_c.transpose(2, 0, 1)], axis=0
        )  # [2D, BL, T]
        arr = np.ascontiguousarray(
            feat.reshape(KT, P, NSB, SB, T).transpose(1, 2, 0, 4, 3)
        ).astype(bf16)  # [P, NSB, KT, T, SB]
        in_maps.append({"inp": arr, "w": w_all, "nbz": nbz_dev, "bh": bh_dev})
    return in_maps


def _finish(results, Wout, bout):
    h_all = np.empty((B, H), np.float32)
    for c in range(NC):
        ho = np.asarray(results[c]["hout"], np.float32)  # [P, HT, BL]
        h_all[c * BL : (c + 1) * BL] = ho.transpose(2, 1, 0).reshape(BL, H)

    wout = np.asarray(Wout, np.float32)
    logits = h_all @ wout[0] + np.asarray(bout, np.float32)[0]
    return (1.0 / (1.0 + np.exp(-logits))).astype(np.float32)


def kernel(X, M, input_means, gamma_x, Wz, bz, Wr, br, Wh, bh, Wout, bout):
    global _last_results
    in_maps = _prepare_in_maps(X, M, input_means, gamma_x, Wz, bz, Wh, bh)
    nc = _build_bass()
    from concourse import bass_utils

    res = bass_utils.run_bass_kernel_spmd(
        nc,
        in_maps,
        core_ids=list(range(NC)),
        trace=False,
    )
    _last_results = res
    return _finish(res.results, Wout, bout)


# revision 8
# speedup vs baseline: 50.2978x; 50.2978x over previous
"""GRU-D forward on 8 Trainium2 NeuronCores (Bass/Tile kernel).

Key algebraic structure exploited:
  - The gates z_t, h~_t depend only on inputs (not on h), so all matmuls are
    parallel over T; only the elementwise blend h = (1-z)h + z*h~ is a
    recurrence, and it maps onto the DVE tensor_tensor_scan instruction
    (state = a*state - c' with a = sigmoid(-pre_z), c' = (a-1)*h~).
  - r_t is computed-but-unused in the reference -> Wr matmul skipped.
  - The xm block of inp = [x_tilde, xm, m] is constant -> folded into biases.

Sharding: data-parallel over batch (64 rows per core); weights replicated.
"""

import os
import sys

import numpy as np

sys.path.insert(0, "/opt/trn_rl_repo")

B, T, D, H = 512, 256, 256, 1024
NC = 8
BL = B // NC  # 64 batch rows per core
SB = 2  # batch elems per sub-batch (x T=256 -> 512 matmul rows)
NSB = BL // SB  # 32 sub-batches
KT = 4  # K tiles of 128 over 2D=512 contraction
HT = 8  # H tiles of 128
P = 128

_nc_cache = {}
_last_results = None


def _build_bass(repeat=1):
    """Build the Bass program. repeat>1 wraps the whole computation in a
    hardware For loop executing it `repeat` times — used only for timing
    (overhead-cancelling slope measurement); the graded path uses repeat=1."""
    if repeat in _nc_cache:
        return _nc_cache[repeat]
    from contextlib import ExitStack

    from concourse import bacc, mybir, tile

    nc = bacc.Bacc("TRN2", target_bir_lowering=False, debug=False, num_devices=NC)
    bf16 = mybir.dt.bfloat16
    f32 = mybir.dt.float32
    AF = mybir.ActivationFunctionType
    OP = mybir.AluOpType

    in_d = nc.dram_tensor("inp", [P, NSB, KT, T, SB], bf16, kind="ExternalInput")
    w_d = nc.dram_tensor("w", [P, 2, KT, HT, P], bf16, kind="ExternalInput")
    nbz_d = nc.dram_tensor("nbz", [P, HT], f32, kind="ExternalInput")
    bh_d = nc.dram_tensor("bh", [P, HT], f32, kind="ExternalInput")
    hout_d = nc.dram_tensor("hout", [P, HT, BL], f32, kind="ExternalOutput")

    with tile.TileContext(nc) as tc:
        with (
            tc.tile_pool(name="const", bufs=1) as cpool,
            tc.tile_pool(name="inb", bufs=3) as ipool,
            tc.tile_pool(name="act", bufs=2) as apool,
            tc.tile_pool(name="zps", bufs=2, space="PSUM") as zpool,
            tc.tile_pool(name="hps", bufs=2, space="PSUM") as hpool,
        ):
            w_s = cpool.tile([P, 2, KT, HT, P], bf16)
            nbz_s = cpool.tile([P, HT], f32)
            bh_s = cpool.tile([P, HT], f32)
            hlast = cpool.tile([P, HT, BL], f32)
            nc.sync.dma_start(w_s[:], w_d[:])
            nc.sync.dma_start(nbz_s[:], nbz_d[:])
            nc.sync.dma_start(bh_s[:], bh_d[:])

            with ExitStack() as rep_ctx:
                if repeat > 1:
                    rep_ctx.enter_context(tc.For_i(0, repeat, 1))
                _emit_body(nc, tc, mybir, ipool, apool, zpool, hpool,
                           in_d, w_s, nbz_s, bh_s, hlast)
            nc.sync.dma_start(hout_d[:], hlast[:])
    nc.compile()
    _nc_cache[repeat] = nc
    return nc


def _emit_body(nc, tc, mybir, ipool, apool, zpool, hpool, in_d, w_s, nbz_s, bh_s, hlast):
    bf16 = mybir.dt.bfloat16
    f32 = mybir.dt.float32
    AF = mybir.ActivationFunctionType
    OP = mybir.AluOpType
    if True:
        if True:
            for sb in range(NSB):
                in_s = ipool.tile([P, KT, T, SB], bf16, tag="in")
                nc.sync.dma_start(in_s[:], in_d[:, sb])
                a_s = apool.tile([P, HT, T, SB], bf16, tag="a")
                ht_s = apool.tile([P, HT, T, SB], bf16, tag="ht")
                cp_s = apool.tile([P, HT, T, SB], bf16, tag="cp")
                so_s = apool.tile([P, HT, T, SB], f32, tag="so")
                for j in range(HT):
                    ps_z = zpool.tile([P, T, SB], f32, tag="z")
                    ps_h = hpool.tile([P, T, SB], f32, tag="h")
                    for k in range(KT):
                        nc.tensor.matmul(
                            ps_z[:],
                            w_s[:, 0, k, j, :],
                            in_s[:, k],
                            start=(k == 0),
                            stop=(k == KT - 1),
                        )
                    for k in range(KT):
                        nc.tensor.matmul(
                            ps_h[:],
                            w_s[:, 1, k, j, :],
                            in_s[:, k],
                            start=(k == 0),
                            stop=(k == KT - 1),
                        )
                    # a = 1 - z = sigmoid(-(pre_z + bz))
                    nc.scalar.activation(
                        a_s[:, j], ps_z[:], AF.Sigmoid, bias=nbz_s[:, j : j + 1], scale=-1.0
                    )
                    nc.scalar.activation(
                        ht_s[:, j], ps_h[:], AF.Tanh, bias=bh_s[:, j : j + 1], scale=1.0
                    )
                    # c' = (a - 1) * h~   (so that a*h - c' = a*h + (1-a)*h~)
                    nc.vector.scalar_tensor_tensor(
                        cp_s[:, j], a_s[:, j], 1.0, ht_s[:, j], op0=OP.subtract, op1=OP.mult
                    )
                    for b in range(SB):
                        nc.vector.tensor_tensor_scan(
                            so_s[:, j, :, b],
                            a_s[:, j, :, b],
                            cp_s[:, j, :, b],
                            0.0,
                            op0=OP.mult,
                            op1=OP.subtract,
                        )
                    nc.vector.tensor_copy(
                        hlast[:, j, sb * SB : (sb + 1) * SB], so_s[:, j, T - 1, :]
                    )


def _prepare_in_maps(X, M, input_means, gamma_x, Wz, bz, Wh, bh):
    import ml_dtypes

    bf16 = ml_dtypes.bfloat16
    X = np.asarray(X, np.float32)
    M = np.asarray(M, np.float32)
    xm = np.asarray(input_means, np.float32)
    gx = np.asarray(gamma_x, np.float32)
    Wz = np.asarray(Wz, np.float32)
    Wh = np.asarray(Wh, np.float32)
    bz = np.asarray(bz, np.float32)
    bhv = np.asarray(bh, np.float32)

    # x_tilde (exact, fp32, handles arbitrary gamma_x / non-binary M)
    g = np.exp(-gx * (1.0 - M))
    x_hat = M * X + (1.0 - M) * xm
    x_tilde = g * x_hat + (1.0 - g) * xm  # [B, T, D]

    # fold the constant xm block into the biases; drop unused Wr entirely
    Wz_eff = np.concatenate([Wz[:, :D], Wz[:, 2 * D :]], axis=1).T  # [2D, H]
    Wh_eff = np.concatenate([Wh[:, :D], Wh[:, 2 * D :]], axis=1).T
    bz_eff = bz + xm @ Wz[:, D : 2 * D].T
    bh_eff = bhv + xm @ Wh[:, D : 2 * D].T

    def wdev(weff):  # [2D, H] -> [P, KT, HT, P]
        return weff.reshape(KT, P, HT, P).transpose(1, 0, 2, 3)

    w_all = np.ascontiguousarray(
        np.stack([wdev(Wz_eff), wdev(Wh_eff)], axis=1)
    ).astype(bf16)  # [P, 2, KT, HT, P]
    nbz_dev = np.ascontiguousarray((-bz_eff).reshape(HT, P).T).astype(np.float32)
    bh_dev = np.ascontiguousarray(bh_eff.reshape(HT, P).T).astype(np.float32)

    in_maps = []
    for c in range(NC):
        xt_c = x_tilde[c * BL : (c + 1) * BL]  # [BL, T, D]
        m_c = M[c * BL : (c + 1) * BL]
        feat = np.concatenate(
            [xt_c.transpose(2, 0, 1), m_c.transpose(2, 0, 1)], axis=0
        )  # [2D, BL, T]
        arr = np.ascontiguousarray(
            feat.reshape(KT, P, NSB, SB, T).transpose(1, 2, 0, 4, 3)
        ).astype(bf16)  # [P, NSB, KT, T, SB]
        in_maps.append({"inp": arr, "w": w_all, "nbz": nbz_dev, "bh": bh_dev})
    return in_maps


def _finish(results, Wout, bout):
    h_all = np.empty((B, H), np.float32)
    for c in range(NC):
        ho = np.asarray(results[c]["hout"], np.float32)  # [P, HT, BL]
        h_all[c * BL : (c + 1) * BL] = ho.transpose(2, 1, 0).reshape(BL, H)

    wout = np.asarray(Wout, np.float32)
    logits = h_all @ wout[0] + np.asarray(bout, np.float32)[0]
    return (1.0 / (1.0 + np.exp(-logits))).astype(np.float32)


def kernel(X, M, input_means, gamma_x, Wz, bz, Wr, br, Wh, bh, Wout, bout):
    global _last_results
    in_maps = _prepare_in_maps(X, M, input_means, gamma_x, Wz, bz, Wh, bh)
    nc = _build_bass()
    from concourse import bass_utils

    res = bass_utils.run_bass_kernel_spmd(
        nc,
        in_maps,
        core_ids=list(range(NC)),
        trace=False,
    )
    _last_results = res
    return _finish(res.results, Wout, bout)
